# revision 31
# baseline (speedup 1.0000x reference)
"""Bass/Tile TRN2 kernel for additive-attention pooling.

Math per sample s:
    e = tanh(x[s] @ W + b)          # (T, 1)
    a = softmax(e, axis=0)          # over T
    y[s] = sum_t a[t] * x[s, t, :]  # (U,)

tanh is bounded in (-1, 1), so softmax needs no max-subtraction:
    p = exp(e);  y[s] = (sum_t p[t] x[s,t]) / (sum_t p[t])

Sharding: data-parallel over batch across 8 NeuronCores (32 samples each).

Per-core dataflow (v2, bf16-cast streaming). The f32 kernel was a
DMA/DVE/ACT triple-point: the x stream ran the 16 SDMA engines at
their full 27 GiB/s (433 GB/s aggregate) but DVE (9.65us/sample) and
ACT (9.63) sat right at the 10.3us/sample DMA budget, causing ~10 x
~1us buffer-recycle stalls mid-stream and a 29us serialized drain
(DVE backlog -> 4 samples of PE back-to-back after the last byte).

v2 keeps the same HBM traffic (x must be read once in f32: 4 MiB per
sample, q-packed so partition p gets 16 contiguous rows = one 32 KiB
chunk) but casts to bf16 in the DMA datapath via SWDGE
(nc.gpsimd.dma_start is the only cast-capable DMA path). Effects:
  - SBUF tiles halve (16 KiB/partition-sample) -> xp pool deepens.
  - DVE score ops hit 2x_1P packed mode: pair tensor_mul [128,1024]
    ~0.69us, fused scalar_tensor_tensor+accum singles ~0.42-0.69us
    -> ~6.3us/sample (was 9.65).
  - PE weighted-sum matmuls stream bf16 moving data at 2 cols/cycle
    -> ~2.7us/sample (was 4.5).
  - ACT: n_pair=3 pairs reduced on ACT (accumulating copy), rest
    fused on DVE -> ~5.8us/sample (was 9.63).
Every compute engine now has >=35% headroom under the DMA stream, so
the stream never stalls on buffer recycling and the drain collapses
to the last sample's dependency chain. bf16 rounding costs ~4e-4 rel
err vs the 2e-2 gate (scores accumulate in fp32 inside DVE/ACT; the
softmax denominator accumulates the f32 exp via accum_out).

Steady-state DVE ops are all single-port (2x_1P packed or 1x): the
per-sample p_sc tail memset was hoisted into setup (4 persistent
ping-pong tiles), so DVE never enters 2-port mode mid-stream --
2-port DVE locks GpSimd out of the SBUF state SWDGE needs for
descriptor emission.

Weighted sum on PE, 4-way column-tiled (slice q -> col group q%4); the
first matmul of each col group uses a 32-wide stationary (its real
column + the persistent zeroed tail) so the whole PSUM bank is defined
without a memzero. Finale batched per 8 samples: exp's accum_out drops
row-sums into column s%8 of a shared rs8 tile, so ONE denominator
matmul, ONE reciprocal, ONE scaled PSUM->SBUF copy and ONE 16 KiB
output DMA serve 8 samples (sel8_j selector matmuls route each
sample's combined row to PSUM partition s%8). First/last two samples
load as two half-DMAs so their score ops overlap the second half's
transfer (subtile deps), trimming pipeline fill and drain.

Hybrid stream split (v3): 14 of 16 slices ride SWDGE as bf16, the
last 2 ride the HWDGE sync ring as f32 (HWDGE descriptors are
RTL-generated - no SBUF descriptor ring - so an SWDGE-fetch-starved
SDMA engine still streams the HWDGE share; fused-single score ops are
1x-mode either way, so the f32 slices cost DVE/ACT nothing). Measured
356.4us with uniform engines immediately after the pure-SWDGE version
measured 417x2 with engine 15 starved.

Pure-SWDGE predecessor measured 344.7 / 353.4 us healthy, vs
368-375 for the f32 HWDGE predecessor. Two device pathologies inflate
either kernel run-to-run: (A) SDMA engine 15 descriptor starvation
(SWDGE descriptor-ring AXI port contention; +60us on the stream;
probability appears build-sensitive - see the dma_start comment in the
loop) and (B) uniform ~19% device-wide slowdown that hits HWDGE f32
just as hard (428us) - in BOTH states this kernel measured faster than
the f32 version, which is why it ships.
"""

from contextlib import ExitStack

import numpy as np

B, T, U = 256, 2048, 512
N_CORES = 8
B_LOC = B // N_CORES
P = 128

_BUILD_CACHE = {}


def _emit(ctx, tc, x, W, b, y, xbufs, n_pair, with_bias):
    from concourse import mybir

    nc = tc.nc
    f32 = mybir.dt.float32
    bf16 = mybir.dt.bfloat16
    Alu = mybir.AluOpType
    Act = mybir.ActivationFunctionType

    b_loc, t_len, u = x.shape
    Q = t_len // P            # rows per partition per sample (16)

    const = ctx.enter_context(tc.tile_pool(name="const", bufs=1))
    xp = ctx.enter_context(tc.tile_pool(name="xp", bufs=xbufs))
    xfp = ctx.enter_context(tc.tile_pool(name="xfp", bufs=xbufs))
    scr = ctx.enter_context(tc.tile_pool(name="scr", bufs=4))
    scr_a = ctx.enter_context(tc.tile_pool(name="scr_a", bufs=2))
    ep = ctx.enter_context(tc.tile_pool(name="ep", bufs=4))
    sp = ctx.enter_context(tc.tile_pool(name="sp", bufs=4))
    op = ctx.enter_context(tc.tile_pool(name="op", bufs=2))
    ps_wb = ctx.enter_context(tc.tile_pool(name="ps_wb", bufs=1, space="PSUM"))
    ps_w = ctx.enter_context(tc.tile_pool(name="ps_w", bufs=4, space="PSUM"))
    ps_s = ctx.enter_context(tc.tile_pool(name="ps_s", bufs=1, space="PSUM"))
    ps_b = ctx.enter_context(tc.tile_pool(name="ps_b", bufs=2, space="PSUM"))

    # ---- constants ----
    # W as a [1, U] row, broadcast to all 128 partitions via a K=1 matmul,
    # then cast to bf16 and duplicated side by side so pair ops can read
    # 2*U of weights. (const loads go on the ACT HWDGE ring so the SWDGE
    # ring's first instruction is sample 0's big load)
    w_row = const.tile([1, u], f32)
    nc.scalar.dma_start(w_row[:], W.rearrange("u o -> o u"))
    ones_row = const.tile([1, P], f32)
    nc.vector.memset(ones_row[:], 1.0)
    ones_col = const.tile([P, 1], f32)
    nc.vector.memset(ones_col[:], 1.0)
    wb_ps = ps_wb.tile([P, u], f32, tag="wb_ps")
    nc.tensor.matmul(wb_ps[:], ones_row[:], w_row[:], start=True, stop=True)
    Wb2 = const.tile([P, 2 * u], bf16)
    nc.vector.tensor_copy(Wb2[:, 0:u], wb_ps[:])
    nc.vector.tensor_copy(Wb2[:, u:2 * u], wb_ps[:])
    # f32 weight row for the two HWDGE-loaded slices (see xf below)
    Wf = const.tile([P, u], f32)
    nc.vector.tensor_copy(Wf[:], wb_ps[:])
    # sel8 block j ([P, 8]): 1.0 at (partition 32i, col j) — sums the 4
    # col-group partial rows of sample s (s%8==j) onto PSUM partition j.
    # One tile holds all 8 blocks (a bufs=1 pool would alias 8 same-shape
    # tiles and deadlock the schedule).
    sel8_t = const.tile([P, 8 * 8], f32)
    nc.vector.memset(sel8_t[:], 0.0)
    for j in range(8):
        for i in range(4):
            nc.vector.memset(sel8_t[32 * i:32 * i + 1, 8 * j + j:8 * j + j + 1], 1.0)
    sel8 = [sel8_t[:, 8 * j:8 * (j + 1)] for j in range(8)]
    # persistent exp-score tiles: zeroed 32-col stationary tail written
    # once here, so the per-sample DVE memset (a 2-port op that fights
    # SWDGE for SBUF) disappears. 4-slot rotation gives the PE matmuls
    # of sample s-4 ample time before exp(s) rewrites the slot.
    p_scs = [const.tile([P, Q + 32], bf16, name=f"p_sc{i}", tag=f"p_sc{i}")
             for i in range(4)]
    for t_ in p_scs:
        nc.vector.memset(t_[:, Q:Q + 32], 0.0)
    if with_bias:
        bt2 = const.tile([P, Q], f32)
        nc.scalar.dma_start(bt2[:], b.rearrange("(p q) o -> p (q o)",
                                                p=P, q=Q))

    xr = x.rearrange("s (p q) u -> s p (q u)", p=P, q=Q)
    QB = Q - 2              # slices streamed via SWDGE as bf16

    def emit_scores(xt, e_sc, q_lo, q_hi):
        # pair slices (q < 2*n_pair): DVE multiplies 2 slices in one
        # packed bf16 op; ACT reduces each 512-block with an accumulating
        # copy (dtype-independent rate; accumulation is internal fp32).
        # single slices: fused multiply+reduce on DVE (fp32 accumulate).
        q = q_lo
        while q < q_hi:
            if q + 1 < min(q_hi, 2 * n_pair):
                pr = scr.tile([P, 2 * u], bf16, name="pr")
                nc.vector.tensor_mul(pr[:], xt[:, q * u:(q + 2) * u], Wb2[:])
                for k in range(2):
                    red = scr_a.tile([P, u], f32, name="red")
                    nc.scalar.activation(red[:], pr[:, k * u:(k + 1) * u],
                                         Act.Copy,
                                         accum_out=e_sc[:, q + k:q + k + 1])
                q += 2
            else:
                sc = scr.tile([P, u], bf16, name="sc")
                nc.vector.scalar_tensor_tensor(
                    out=sc[:],
                    in0=xt[:, q * u:(q + 1) * u],
                    scalar=1.0,
                    in1=Wb2[:, 0:u],
                    op0=Alu.mult,
                    op1=Alu.mult,
                    accum_out=e_sc[:, q:q + 1],
                )
                q += 1

    def emit_wsum(wsum, p_sc, pf, xt, xf, q_lo, q_hi, wide_first):
        # weighted sum: slice q -> col group q%4, partial at psum row 32j.
        # wide_first: the q<4 matmuls use a 32-wide stationary (real col +
        # persistent zero tail) so each whole 32-row PSUM group is defined
        # without a memzero; requires p_sc[:, q:q+32] fully written, i.e.
        # the un-chunked path only.
        for q in range(q_lo, q_hi):
            j = q % 4
            if wide_first and q < 4:
                nc.tensor.matmul(
                    wsum[32 * j:32 * j + 32, :],
                    p_sc[:, q:q + 32],
                    xt[:, q * u:(q + 1) * u],
                    start=True, stop=False,
                    tile_position=(0, 32 * j),
                )
            else:
                # PE requires matching dtypes: bf16 slices use the bf16
                # p_sc column, f32 (HWDGE) slices use the f32 copy pf
                if q < QB:
                    st, mv = p_sc[:, q:q + 1], xt[:, q * u:(q + 1) * u]
                else:
                    st = pf[:, q - QB:q - QB + 1]
                    mv = xf[:, (q - QB) * u:(q - QB + 1) * u]
                # un-wide path accumulates onto a pre-zeroed bank
                nc.tensor.matmul(
                    wsum[32 * j:32 * j + 1, :],
                    st,
                    mv,
                    start=False,
                    stop=(q >= Q - 4),
                    tile_position=(0, 32 * j),
                )
    for s in range(b_loc):
        g = s % 8
        # fat tile: most of the sample, partition p holds QB contiguous
        # HBM rows, cast f32->bf16 in the SDMA datapath (SWDGE). The last
        # 2 slices ride the HWDGE (sync) ring as f32 into xf: HWDGE
        # descriptors are RTL-generated (no SBUF descriptor ring), so when
        # SDMA engine 15 is starved of SWDGE descriptor fetches (pathology
        # A) it can still stream the HWDGE share; fused-single score ops
        # are 1x-mode regardless of dtype, so the f32 slices cost DVE/ACT
        # nothing extra.
        xt = xp.tile([P, QB * u], bf16)
        xf = xfp.tile([P, 2 * u], f32)
        wsum = ps_w.tile([P, u], f32, tag="wsum")

        if g == 0:
            rs8 = sp.tile([P, 8], f32, tag="rs8")
            bat = ps_b.tile([8, u], f32, tag="bat")

        e_sc = ep.tile([P, Q], f32, tag="e_sc")
        p_sc = p_scs[s % 4]

        # First and last two samples load the bf16 share as two half-DMAs:
        # scores on the first half overlap the second half's transfer
        # (subtile deps), trimming pipeline fill and drain. The OTHER
        # samples use one full SWDGE DMA each — keep the SWDGE dma_start
        # count minimal (descriptor-ring pressure, pathology A).
        if s == 0 or s >= b_loc - 2:
            hw = (QB // 2) * u
            nc.gpsimd.dma_start(xt[:, 0:hw], xr[s][:, 0:hw])
            nc.gpsimd.dma_start(xt[:, hw:], xr[s][:, hw:QB * u])
        else:
            nc.gpsimd.dma_start(xt[:], xr[s][:, 0:QB * u])
        nc.sync.dma_start(xf[:], xr[s][:, QB * u:])
        # f32 (HWDGE) slices' scores FIRST: xf lands ~1us after issue
        # while the SWDGE bulk takes ~10us, so putting these early keeps
        # DVE's in-order tail pointed at the last-arriving bf16 data —
        # shortens the end-of-stream drain by the two ops' time.
        for q in range(QB, Q):
            scf = scr.tile([P, u], f32, name="scf")
            nc.vector.scalar_tensor_tensor(
                out=scf[:],
                in0=xf[:, (q - QB) * u:(q - QB + 1) * u],
                scalar=1.0,
                in1=Wf[:],
                op0=Alu.mult,
                op1=Alu.mult,
                accum_out=e_sc[:, q:q + 1],
            )
        emit_scores(xt, e_sc, 0, QB)
        if with_bias:
            eb_sc = ep.tile([P, Q], f32, tag="eb_sc")
            nc.vector.tensor_add(eb_sc[:], e_sc[:], bt2[:])
        else:
            eb_sc = e_sc
        th_sc = ep.tile([P, Q], f32, tag="th_sc")
        nc.scalar.activation(th_sc[:], eb_sc[:], Act.Tanh)
        # exp into the persistent slot (bf16 for the PE stationary);
        # accum_out drops the f32 row-sums into column g of rs8.
        nc.scalar.activation(p_sc[:, 0:Q], th_sc[:], Act.Exp,
                             accum_out=rs8[:, g:g + 1])
        # f32 copy of the two HWDGE slices' exp weights (PE needs the
        # stationary dtype to match the f32 moving tensor)
        pf = ep.tile([P, 2], f32, tag="pf")
        nc.scalar.activation(pf[:], p_sc[:, QB:Q], Act.Copy)
        emit_wsum(wsum, p_sc, pf, xt, xf, 0, Q, wide_first=True)

        # combine the 4 partial rows and route to batch-PSUM partition g
        wsb = op.tile([P, u], f32, tag="wsb")
        nc.scalar.activation(wsb[:], wsum[:], Act.Copy)
        nc.tensor.matmul(bat[:], sel8[g], wsb[:],
                         start=(g == 0), stop=(g == 7))

        if g == 7:
            # denominators for the whole 8-group in one matmul
            s_ps = ps_s.tile([8, 1], f32)
            nc.tensor.matmul(s_ps[:], rs8[:], ones_col[:],
                             start=True, stop=True)
            inv8 = sp.tile([8, 1], f32, tag="inv8")
            nc.vector.reciprocal(inv8[:], s_ps[:])
            obat = op.tile([8, u], f32, tag="obat")
            nc.scalar.activation(obat[:], bat[:], Act.Copy, scale=inv8[:])
            nc.sync.dma_start(y[s - 7:s + 1, :], obat[:])


def build_nc(b_loc=B_LOC, t_len=T, u=U, xbufs=8, n_pair=3, with_bias=False,
             dma_scratch=16384):
    key = (b_loc, t_len, u, xbufs, n_pair, with_bias, dma_scratch)
    if key in _BUILD_CACHE:
        return _BUILD_CACHE[key]
    import concourse.bacc as bacc
    import concourse.tile as tile
    from concourse import mybir

    nc = bacc.Bacc(
        "TRN2",
        target_bir_lowering=False,
        debug=False,
        num_devices=N_CORES,
        dynamic_dma_scratch_size=dma_scratch,
    )
    x = nc.dram_tensor("x", [b_loc, t_len, u], mybir.dt.float32, kind="ExternalInput").ap()
    W = nc.dram_tensor("W", [u, 1], mybir.dt.float32, kind="ExternalInput").ap()
    b = nc.dram_tensor("b", [t_len, 1], mybir.dt.float32, kind="ExternalInput").ap()
    y = nc.dram_tensor("y", [b_loc, u], mybir.dt.float32, kind="ExternalOutput").ap()

    with tile.TileContext(nc) as tc:
        with ExitStack() as ctx:
            _emit(ctx, tc, x, W, b, y, xbufs, n_pair, with_bias)
    nc.compile()
    _BUILD_CACHE[key] = nc
    return nc


def kernel(x, W, b):
    x = np.ascontiguousarray(np.asarray(x, dtype=np.float32))
    W = np.ascontiguousarray(np.asarray(W, dtype=np.float32))
    b = np.ascontiguousarray(np.asarray(b, dtype=np.float32))
    assert x.shape == (B, T, U), x.shape

    from concourse.bass_utils import run_bass_kernel_spmd

    # b is all-zero in this problem's setup; the fast path skips the bias
    # add on the (bottleneck) DVE. A general build handles nonzero b.
    nc = build_nc(with_bias=bool(np.any(b)))
    in_maps = [
        {
            "x": np.ascontiguousarray(x[i * B_LOC:(i + 1) * B_LOC]),
            "W": W,
            "b": b,
        }
        for i in range(N_CORES)
    ]
    res = run_bass_kernel_spmd(nc, in_maps, core_ids=list(range(N_CORES)))
    return np.concatenate([r["y"] for r in res.results], axis=0)
